# revision 15
# baseline (speedup 1.0000x reference)
"""Spatial self-attention (SAGAN-style) kernel for 8 Trainium2 NeuronCores.

Math (per batch b):
    xf  = x[b].reshape(C, N)                       # C=256, N=4096
    qT  = wq @ xf + bq                             # [32, N]
    kT  = wk @ xf + bk                             # [32, N]
    V   = wv @ xf + bv                             # [C, N]
    E^T = kT.T @ qT                                # [m, n]  (keys on partitions)
    A'  = exp(E^T)          (no max-subtraction: |E| < 29, safe in fp32)
    s   = colsum(A')                               # softmax denominator per query
    out = gamma * (V @ A / s) + x

Sharding: core i handles batch b = i//2, query half h = i%2 (2048 queries).
Each core computes kT / V^T for the full 4096 keys of its batch. The host
rotates xf per-core so the core's 2048 query columns always sit at columns
0..2047 (attention reductions are permutation-invariant over keys), which
keeps the SPMD program uniform with no separate query tensor.

Device layout choices:
  - E^T orientation (keys on PSUM partitions) so A' feeds the output matmul
    as the moving operand with zero transposes anywhere.
  - V^T [m, c] computed directly on PE (lhsT = xf block, rhs = (g*wv).T).
  - softmax denominator via an all-ones [128,128] stationary matmul that
    accumulates alongside the two output-channel matmuls -> s replicated
    across all 128 partitions for free.
  - gamma folded into wv/bv on the host; residual added on DVE.
  - all matmuls in float32r (single-pass fp32, 1 cycle/row at N>=256); the
    BIR verifier requires every fp32r-matmul operand's memory to be written
    only by rounding-capable compute ops, so DMA loads land in fp32 staging
    tiles and DVE copies produce the fp32r tiles.
  - walrus allows at most ONE semaphore wait on a (self-loading fp32r)
    Matmult, so: every matmul input is produced on DVE, the O accumulators
    are read back on ACT only, and all pools live for the whole kernel
    (pool release zones would otherwise add extra waits); each matmul's
    cross-engine deps then collapse to a single engine semaphore.
"""

import numpy as np

import concourse.bass as bass
import concourse.mybir as mybir
import concourse.tile as tile
from concourse.bass import ts
from concourse.bass_utils import run_bass_kernel_spmd

B, C, HH, WW = 4, 256, 64, 64
N = HH * WW          # 4096 spatial positions
D = 32               # C // 8 head dim
NCORES = 8
NQ = N * B // NCORES  # 2048 queries per core
MB = N // 128        # 32 key blocks
QCH = NQ // 512      # 4 query chunks per core
KCH = N // 512       # 8 chunks across keys

F32 = mybir.dt.float32
F32R = mybir.dt.float32r
AF = mybir.ActivationFunctionType
OP = mybir.AluOpType


def _build():
    nc = bass.Bass()
    xf = nc.declare_dram_parameter("xf", [C, N], F32, isOutput=False)
    wqT = nc.declare_dram_parameter("wqT", [C, D], F32, isOutput=False)
    wkT = nc.declare_dram_parameter("wkT", [C, D], F32, isOutput=False)
    wvT = nc.declare_dram_parameter("wvT", [C, C], F32, isOutput=False)
    bq4 = nc.declare_dram_parameter("bq4", [128, 1], F32, isOutput=False)
    bk4 = nc.declare_dram_parameter("bk4", [128, 1], F32, isOutput=False)
    bvr = nc.declare_dram_parameter("bvr", [128, C], F32, isOutput=False)
    out = nc.declare_dram_parameter("out", [C, NQ], F32, isOutput=True)

    with tile.TileContext(nc) as tc:
        with (
            tc.tile_pool(name="const", bufs=1) as constp,
            tc.tile_pool(name="xfp", bufs=1) as xfp,
            tc.tile_pool(name="big", bufs=1) as bigp,
            tc.tile_pool(name="apool", bufs=3) as apool,
            tc.tile_pool(name="fin", bufs=2) as finp,
            tc.tile_pool(name="psO", bufs=1, space="PSUM") as psO,
            tc.tile_pool(name="psE", bufs=4, space="PSUM") as psE,
        ):
            xf_s = [xfp.tile([128, N], F32, name=f"xfs{i}") for i in range(2)]
            xf_t = [xfp.tile([128, N], F32R, name=f"xf{i}") for i in range(2)]
            wq_s = [constp.tile([128, D], F32, name=f"wqs{i}") for i in range(2)]
            wk_s = [constp.tile([128, D], F32, name=f"wks{i}") for i in range(2)]
            wv_s = [constp.tile([128, C], F32, name=f"wvs{i}") for i in range(2)]
            wq_t = [constp.tile([128, D], F32R, name=f"wq{i}") for i in range(2)]
            wk_t = [constp.tile([128, D], F32R, name=f"wk{i}") for i in range(2)]
            wv_t = [constp.tile([128, C], F32R, name=f"wv{i}") for i in range(2)]
            bq_s = constp.tile([128, 1], F32, name="bqs")
            bk_s = constp.tile([128, 1], F32, name="bks")
            bv_s = constp.tile([128, C], F32, name="bvs")
            bq_t = constp.tile([128, 1], F32, name="bq")
            bk_t = constp.tile([128, 1], F32, name="bk")
            bv_t = constp.tile([128, C], F32, name="bv")
            ones_s = constp.tile([128, 128], F32, name="ones_s")
            ones_t = constp.tile([128, 128], F32R, name="ones")
            # kT/qT live in rows 0:32; vT block m occupies cols [m*C, (m+1)*C)
            kT = bigp.tile([128, N], F32R, name="kT")
            qT = bigp.tile([128, NQ], F32R, name="qT")
            vT = bigp.tile([128, MB * C], F32R, name="vT")

            # DMA into fp32 staging, DVE-copy into the fp32r matmul operands
            # (so matmuls only ever wait on the DVE semaphore).
            for st_, tl, src in (
                    (xf_s[0], xf_t[0], xf[0:128, :]),
                    (xf_s[1], xf_t[1], xf[128:256, :]),
                    (wq_s[0], wq_t[0], wqT[0:128, :]),
                    (wq_s[1], wq_t[1], wqT[128:256, :]),
                    (wk_s[0], wk_t[0], wkT[0:128, :]),
                    (wk_s[1], wk_t[1], wkT[128:256, :]),
                    (wv_s[0], wv_t[0], wvT[0:128, :]),
                    (wv_s[1], wv_t[1], wvT[128:256, :])):
                nc.sync.dma_start(st_[:], src)
                nc.vector.tensor_copy(tl[:], st_[:])
            nc.sync.dma_start(bq_s[:], bq4[:, :])
            nc.sync.dma_start(bk_s[:], bk4[:, :])
            nc.sync.dma_start(bv_s[:], bvr[:, :])
            nc.vector.tensor_copy(bq_t[:], bq_s[:])
            nc.vector.tensor_copy(bk_t[:], bk_s[:])
            nc.vector.tensor_copy(bv_t[:], bv_s[:])
            nc.vector.memset(ones_s[:], 1.0)
            nc.vector.tensor_copy(ones_t[:], ones_s[:])

            # phase 1: qT, kT, vT projections (PSUM extracted on DVE); the
            # PSUM tiles share the "e" tag with phase 2's E tiles so slot
            # reuse stays a DVE-sem WAR dep.
            for ch in range(QCH):
                ps = psE.tile([D, 512], F32, tag="e", name=f"psq{ch}")
                for cb in range(2):
                    nc.tensor.matmul(
                        ps[:],
                        lhsT=wq_t[cb][:],
                        rhs=xf_t[cb][:, ts(ch, 512)],
                        start=(cb == 0), stop=(cb == 1),
                    )
                nc.vector.tensor_scalar_add(qT[0:D, ts(ch, 512)], ps[:],
                                            bq_t[0:D, 0:1])
            for ch in range(KCH):
                ps = psE.tile([D, 512], F32, tag="e", name=f"psk{ch}")
                for cb in range(2):
                    nc.tensor.matmul(
                        ps[:],
                        lhsT=wk_t[cb][:],
                        rhs=xf_t[cb][:, ts(ch, 512)],
                        start=(cb == 0), stop=(cb == 1),
                    )
                nc.vector.tensor_scalar_add(kT[0:D, ts(ch, 512)], ps[:],
                                            bk_t[0:D, 0:1])
            for m in range(MB):
                psv = psE.tile([128, C], F32, tag="e", name=f"psv{m}")
                for cb in range(2):
                    nc.tensor.matmul(
                        psv[:],
                        lhsT=xf_t[cb][:, ts(m, 128)],
                        rhs=wv_t[cb][:],
                        start=(cb == 0), stop=(cb == 1),
                    )
                nc.vector.tensor_tensor(vT[:, ts(m, C)], psv[:], bv_t[:], OP.add)

            # phase 2: E^T -> exp -> V@A + colsum, one 512-query chunk at a time
            for ch in range(QCH):
                oc = [psO.tile([128, 512], F32, tag=f"oc{j}", name=f"oc{j}_{ch}")
                      for j in range(3)]
                for g in range(MB // 4):
                    ats = []
                    for i in range(4):
                        m = 4 * g + i
                        e = psE.tile([128, 512], F32, tag="e", name=f"e{ch}_{m}")
                        nc.tensor.matmul(
                            e[:],
                            lhsT=kT[0:D, ts(m, 128)],
                            rhs=qT[0:D, ts(ch, 512)],
                            start=True, stop=True, skip_group_check=True,
                        )
                        a = apool.tile([128, 512], F32R, tag="a", name=f"a{ch}_{m}")
                        nc.scalar.activation(a[:], e[:], AF.Exp)
                        ats.append(a)
                    for i in range(4):
                        m = 4 * g + i
                        st, sp = (m == 0), (m == MB - 1)
                        nc.tensor.matmul(
                            oc[0][:], lhsT=vT[:, m * C:m * C + 128],
                            rhs=ats[i][:], start=st, stop=sp, skip_group_check=True)
                        nc.tensor.matmul(
                            oc[1][:], lhsT=vT[:, m * C + 128:(m + 1) * C],
                            rhs=ats[i][:], start=st, stop=sp, skip_group_check=True)
                        nc.tensor.matmul(
                            oc[2][:], lhsT=ones_t[:],
                            rhs=ats[i][:], start=st, stop=sp, skip_group_check=True)
                # read the three accumulator banks back on ACT so the next
                # chunk's matmuls see only the ACT semaphore as WAR dep
                o_sb = [finp.tile([128, 512], F32, tag=f"osb{j}", bufs=4,
                                  name=f"osb{j}_{ch}") for j in range(3)]
                for j in range(3):
                    nc.scalar.copy(o_sb[j][:], oc[j][:])
                r = finp.tile([128, 512], F32, tag="r", name=f"r{ch}")
                nc.vector.reciprocal(r[:], o_sb[2][:])
                for cb in range(2):
                    t = finp.tile([128, 512], F32, tag="t", name=f"t{ch}_{cb}")
                    nc.vector.tensor_tensor(t[:], o_sb[cb][:], r[:], OP.mult)
                    f = finp.tile([128, 512], F32, tag="f", bufs=4,
                                  name=f"f{ch}_{cb}")
                    nc.vector.tensor_tensor(f[:], t[:],
                                            xf_s[cb][:, ts(ch, 512)],
                                            OP.add)
                    nc.gpsimd.dma_start(out[cb * 128:(cb + 1) * 128, ts(ch, 512)],
                                        f[:])
    _strip_self_waits(nc)
    _split_multi_waits(nc)
    return nc


_ENGINE_SEM_PREFIX = {
    "EngineType.PE": "PE_",
    "EngineType.DVE": "DVE_",
    "EngineType.Activation": "Activation_",
    "EngineType.Pool": "Pool_",
    "EngineType.SP": "SP_",
}


def _strip_self_waits(nc):
    """Drop same-engine semaphore waits from multi-wait TPB instructions.

    Walrus allows exactly one sync wait per TPB instruction. Tile emits
    redundant self-engine waits (WAW on pool-slot reuse, RAW from same-engine
    producers): each engine executes its queue in order, so a wait on the
    engine's own semaphore is always satisfied by program order. Dropping
    them collapses every instruction to at most one (cross-engine) wait.
    """
    for bb in nc.m.functions[0].blocks:
        for inst in bb.instructions:
            si = inst.sync_info
            if si is None:
                continue
            w = si.on_wait
            if len(w) <= 1 or inst.opcode == "Drain":
                continue
            pfx = _ENGINE_SEM_PREFIX.get(str(inst.engine))
            if pfx is None:
                continue
            kept = [x for x in w if not x.ant_name.startswith(pfx)]
            if kept and len(kept) < len(w):
                si.on_wait = kept


def _split_multi_waits(nc):
    """Walrus allows one sync wait per TPB instruction; move surplus waits
    onto dedicated single-wait Drain instructions inserted just before the
    offender (same engine, executes in order)."""
    import bass_rust
    cnt = 0
    for bb in nc.m.functions[0].blocks:
        il = bb.instructions
        i = 0
        while i < len(il):
            inst = il[i]
            si = inst.sync_info
            w = si.on_wait if si else []
            if len(w) > 1:
                for j, wait in enumerate(w[:-1]):
                    d = mybir.InstDrain(name=f"{inst.name}-w{j}", ins=[], outs=[],
                                        bass_is_fusable=False)
                    d.engine = inst.engine
                    d.sync_info = bass_rust.SyncInfo(on_wait=[wait], on_update=[])
                    il.insert(i, d)
                    i += 1
                    cnt += 1
                si.on_wait = [w[-1]]
            i += 1
    return cnt


def audit_matmul_waits(nc):
    """Max sync-wait count on any Matmult (walrus limit: 1)."""
    worst = (0, None)
    for bb in nc.m.functions[0].blocks:
        for inst in bb.instructions:
            if inst.opcode != "Matmult":
                continue
            w = inst.sync_info.on_wait if inst.sync_info else []
            if len(w) > worst[0]:
                worst = (len(w), (inst.name, [x.ant_name for x in w]))
    return worst


_NC_CACHE = None


def _get_nc():
    global _NC_CACHE
    if _NC_CACHE is None:
        _NC_CACHE = _build()
    return _NC_CACHE


def kernel(x, wq, bq, wk, bk, wv, bv, gamma, _trace=False):
    f32 = lambda a: np.ascontiguousarray(np.asarray(a, dtype=np.float32))
    x = f32(x)
    g = float(np.asarray(gamma).reshape(-1)[0])
    xfull = x.reshape(B, C, N)
    shared = {
        "wqT": f32(np.asarray(wq).T),
        "wkT": f32(np.asarray(wk).T),
        "wvT": f32((g * np.asarray(wv)).T),
        "bq4": f32(np.tile(np.asarray(bq).reshape(D, 1), (128 // D, 1))),
        "bk4": f32(np.tile(np.asarray(bk).reshape(D, 1), (128 // D, 1))),
        "bvr": f32(np.tile((g * np.asarray(bv)).reshape(1, C), (128, 1))),
    }
    in_maps = []
    for core in range(NCORES):
        b, h = core // 2, core % 2
        m = dict(shared)
        if h == 0:
            m["xf"] = f32(xfull[b])
        else:
            # rotate so this core's query half sits at columns 0..NQ-1;
            # key order is irrelevant (attention reduces over all keys)
            m["xf"] = f32(np.concatenate(
                [xfull[b][:, NQ:], xfull[b][:, :NQ]], axis=1))
        in_maps.append(m)

    res = run_bass_kernel_spmd(_get_nc(), in_maps, list(range(NCORES)),
                               trace=_trace)
    full = np.empty((B, C, N), np.float32)
    for core in range(NCORES):
        b, h = core // 2, core % 2
        full[b][:, h * NQ:(h + 1) * NQ] = res.results[core]["out"]
    out = full.reshape(B, C, HH, WW)
    if _trace:
        return out, res
    return out
